# revision 1
# baseline (speedup 1.0000x reference)
"""Decoupled InfoNCE loss on 8 Trainium2 NeuronCores (Bass/Tile SPMD).

Math (reference):
    e = x / max(||x||, 1e-8);  sim = (e @ e.T) / 0.1
    pos = (t_i == t_j);  lse_neg = LSE_j(sim | not pos);  lse_pos = LSE_j(sim | pos & j != i)
    loss = sum_i (lse_neg_i - lse_pos_i)

Device strategy (per core c, anchors = rows [c*B, c*B+B)):
  * All logits sim/T lie in [-10, 10], so exp never overflows and the LSE
    max-subtraction can be dropped: lse = log(sum exp(sim/T)).
  * Inputs are row-rotated per core (np.roll) so each core's anchors are rows
    [0, B) of its own copy -> all 8 cores run one identical static program.
  * eT = transpose(sqrt(10) * e) is built on-chip ([d, j] layout, d on
    partitions); sim-chunks [128 j, 512 i] come from two K=128 matmuls.
  * Class masks are rank-64: with onehot tags M[cls, i] = sum_j 1[t_j==cls] *
    exp(sim_ji) (one extra matmul per chunk vs. the exp tile), then
      S_pos_incl[i] = M[t_i, i],  S_neg[i] = sum_cls M[cls, i] - M[t_i, i]
    computed exactly by elementwise one-hot select + ones-matmul column sums.
  * The diagonal exp(sim_ii) is extracted exactly from the exp tiles of the
    first 8 j-blocks (where the diagonal lives after rotation) and subtracted
    from S_pos_incl.
  * Output per core: per-anchor loss rows [B]; host concatenates and sums.
"""

import sys

if "/opt/trn_rl_repo" not in sys.path:
    sys.path.insert(0, "/opt/trn_rl_repo")

import numpy as np

N = 8192          # total rows
D = 256           # embedding dim
C = 64            # num classes
NCORES = 8
B = N // NCORES   # anchors per core
SQT = float(np.sqrt(10.0))  # sqrt(1/temperature); applied to both operands
EPS = 1e-8

_NC_CACHE = {}


def _build_nc(n=N, d=D, ncls=C, ncores=NCORES, reps=1):
    import concourse.bass as bass
    import concourse.mybir as mybir
    from concourse import tile

    f32 = mybir.dt.float32
    Act = mybir.ActivationFunctionType
    AX = mybir.AxisListType

    b = n // ncores       # anchors per core
    nt = n // 128         # j tiles
    hb = d // 128         # 128-deep K steps per matmul
    nab = b // 512        # 512-wide anchor blocks
    ndj = b // 128        # j-blocks containing diagonal (first ndj blocks)

    nc = bass.Bass()
    x_d = nc.dram_tensor("x", [n, d], f32, kind="ExternalInput")
    tag_d = nc.dram_tensor("tag", [n, ncls], f32, kind="ExternalInput")
    oha_d = nc.dram_tensor("oha", [ncls, b], f32, kind="ExternalInput")
    i128_d = nc.dram_tensor("i128", [128, 128], f32, kind="ExternalInput")
    loss_d = nc.dram_tensor("loss", [1, b], f32, kind="ExternalOutput")

    with tile.TileContext(nc) as tc:
        with (
            tc.tile_pool(name="persist", bufs=1) as pp,
            tc.tile_pool(name="work", bufs=6) as wp,
            tc.tile_pool(name="expp", bufs=10) as ep,
            tc.tile_pool(name="c0p", bufs=6, space="PSUM") as c0p,
            tc.tile_pool(name="mp", bufs=1, space="PSUM") as mp,
        ):
            # ---- persistent SBUF ----
            eT = pp.tile([128, hb, n], f32, tag="eT")
            tagS = pp.tile([128, nt, ncls], f32, tag="tagS")
            ohaS = pp.tile([ncls, b], f32, tag="ohaS")
            i128 = pp.tile([128, 128], f32, tag="i128")
            ones = pp.tile([ncls, 1], f32, tag="ones")
            ddrow = pp.tile([1, b], f32, tag="ddrow")    # exp(sim_ii)
            sposr = pp.tile([1, b], f32, tag="sposr")    # S_pos incl diag
            snegr = pp.tile([1, b], f32, tag="snegr")    # S_neg
            lnegr = pp.tile([1, b], f32, tag="lnegr")
            lossr = pp.tile([1, b], f32, tag="lossr")

            nc.sync.dma_start(out=tagS[:], in_=tag_d.rearrange("(t p) c -> p t c", p=128))
            nc.sync.dma_start(out=ohaS[:], in_=oha_d[:])
            nc.sync.dma_start(out=i128[:], in_=i128_d[:])
            nc.vector.memset(ones[:], 1.0)

            # macc psum accumulators live across the whole j loop
            macc = [mp.tile([ncls, 512], f32, tag=f"m{ab}", name=f"macc{ab}")
                    for ab in range(nab)]

            # reps>1 repeats the whole computation in one NEFF; used only to
            # measure per-iteration HW time as a slope (dispatch overhead on
            # the axon path dwarfs a single run).
            for _rep in range(reps):
                _emit_body(nc, tile, mybir, locals())

    _split_multi_waits(nc)
    nc.finalize()
    return nc


def _emit_body(nc, tile, mybir, env):
    f32 = mybir.dt.float32
    Act = mybir.ActivationFunctionType
    AX = mybir.AxisListType
    n, d, ncls, b = env["n"], env["d"], env["ncls"], env["b"]
    nt, hb, nab, ndj = env["nt"], env["hb"], env["nab"], env["ndj"]
    x_d, loss_d = env["x_d"], env["loss_d"]
    eT, tagS, ohaS, i128, ones = (env["eT"], env["tagS"], env["ohaS"],
                                  env["i128"], env["ones"])
    ddrow, sposr, snegr, lnegr, lossr = (env["ddrow"], env["sposr"],
                                         env["snegr"], env["lnegr"],
                                         env["lossr"])
    wp, ep, c0p = env["wp"], env["ep"], env["c0p"]
    macc = env["macc"]

    if True:
        if True:
            # ---- phase 1: normalize + scale + transpose into eT ----
            # Per 128-row tile: inv = sqrt(10)/max(||row||, eps); then
            # eT[:, h, rows] = (x[:, h*128:...].T @ diag(inv)) via matmul with
            # a row-scaled identity (fuses the scale into the transpose).
            for t in range(nt):
                xt = wp.tile([128, d], f32, tag="xt")
                nc.sync.dma_start(out=xt[:], in_=x_d[t * 128:(t + 1) * 128, :])
                sq = wp.tile([128, d], f32, tag="sq")
                nc.vector.tensor_mul(sq[:], xt[:], xt[:])
                ss = wp.tile([128, 1], f32, tag="ss")
                nc.vector.reduce_sum(ss[:], sq[:], axis=AX.X)
                nrm = wp.tile([128, 1], f32, tag="nrm")
                nc.scalar.activation(nrm[:], ss[:], Act.Sqrt)
                nc.vector.tensor_scalar_max(nrm[:], nrm[:], EPS)
                inv = wp.tile([128, 1], f32, tag="inv")
                nc.vector.reciprocal(inv[:], nrm[:])
                dg = wp.tile([128, 128], f32, tag="dg")
                nc.vector.tensor_scalar(dg[:], i128[:], inv[:], SQT,
                                        op0=mybir.AluOpType.mult,
                                        op1=mybir.AluOpType.mult)
                for h in range(hb):
                    pt = c0p.tile([128, 128], f32, tag="c0")
                    nc.tensor.matmul(pt[:], xt[:, h * 128:(h + 1) * 128], dg[:],
                                     start=True, stop=True)
                    nc.vector.tensor_copy(eT[:, h, t * 128:(t + 1) * 128], pt[:])

            # ---- phase 2: sim chunks -> exp -> class-sum matmuls ----
            live = {}  # (jb, ab) -> exp tile, consumed one jb later

            def consume(jb):
                exs = [live.pop((jb, ab)) for ab in range(nab)]
                for ab in range(nab):
                    nc.tensor.matmul(macc[ab][:], tagS[:, jb, :], exs[ab][:],
                                     start=(jb == 0), stop=(jb == nt - 1))
                if jb < ndj:
                    abd, off = (jb * 128) // 512, (jb * 128) % 512
                    dtmp = wp.tile([128, 128], f32, tag="dtmp")
                    nc.vector.tensor_mul(dtmp[:], exs[abd][:, off:off + 128], i128[:])
                    dcol = wp.tile([128, 1], f32, tag="dcol")
                    nc.vector.reduce_sum(dcol[:], dtmp[:], axis=AX.X)
                    drow = c0p.tile([1, 512], f32, tag="c0", name="drow")
                    nc.tensor.matmul(drow[:, :128], dcol[:], i128[:],
                                     start=True, stop=True)
                    nc.vector.tensor_copy(ddrow[:, jb * 128:(jb + 1) * 128], drow[:, :128])

            LAG = 2  # jb's of slack between C0 production and M' consumption
            for jb in range(nt):
                # h-outer order: consecutive matmuls share lhsT (one weight
                # load per (jb, h) instead of per (jb, h, ab)); accumulation
                # groups of the nab psum banks interleave, which the PE
                # handles per-bank (skip the group contiguity check).
                c0s = []
                for ab in range(nab):
                    c0 = c0p.tile([128, 512], f32, tag="c0", name=f"c0_{jb}_{ab}")
                    c0s.append(c0)
                for h in range(hb):
                    for ab in range(nab):
                        nc.tensor.matmul(c0s[ab][:],
                                         eT[:, h, jb * 128:(jb + 1) * 128],
                                         eT[:, h, ab * 512:(ab + 1) * 512],
                                         start=(h == 0), stop=(h == hb - 1),
                                         skip_group_check=True)
                for ab in range(nab):
                    ex = ep.tile([128, 512], f32, tag="exp", name=f"ex_{jb}_{ab}")
                    nc.scalar.activation(ex[:], c0s[ab][:], Act.Exp)
                    live[(jb, ab)] = ex
                if jb >= LAG:
                    consume(jb - LAG)
            for jb in range(nt - LAG, nt):
                consume(jb)

            # ---- phase 3: select own-class / other-class sums ----
            for ab in range(nab):
                msb = wp.tile([ncls, 512], f32, tag="msb")
                nc.vector.tensor_copy(msb[:], macc[ab][:])
                x1 = wp.tile([ncls, 512], f32, tag="x1")
                nc.vector.tensor_mul(x1[:], msb[:], ohaS[:, ab * 512:(ab + 1) * 512])
                x2 = wp.tile([ncls, 512], f32, tag="x2")
                nc.vector.tensor_sub(x2[:], msb[:], x1[:])
                s1 = c0p.tile([1, 512], f32, tag="c0", name="s1")
                nc.tensor.matmul(s1[:], ones[:], x1[:], start=True, stop=True)
                nc.vector.tensor_copy(sposr[:, ab * 512:(ab + 1) * 512], s1[:])
                s2 = c0p.tile([1, 512], f32, tag="c0", name="s2")
                nc.tensor.matmul(s2[:], ones[:], x2[:], start=True, stop=True)
                nc.vector.tensor_copy(snegr[:, ab * 512:(ab + 1) * 512], s2[:])

            # ---- phase 4: loss rows ----
            nc.vector.tensor_sub(sposr[:], sposr[:], ddrow[:])
            nc.scalar.activation(lnegr[:], snegr[:], Act.Ln)
            nc.scalar.activation(lossr[:], sposr[:], Act.Ln)
            nc.vector.tensor_sub(lossr[:], lnegr[:], lossr[:])
            nc.sync.dma_start(out=loss_d[:], in_=lossr[:])


def _split_multi_waits(nc):
    """Move extra semaphore waits onto standalone EventSemaphore carriers.

    The pinned walrus build only has one sync-wait slot per engine
    instruction ("Too many sync wait commands"), while the Tile scheduler
    happily attaches several. All waits here are monotonic sem-ge-imm, so
    waiting sequentially on the same engine is equivalent to waiting on the
    conjunction.
    """
    import concourse.mybir as mybir

    for fn in nc.m.functions:
        for blk in fn.blocks:
            out = []
            for inst in blk.instructions:
                si = inst.sync_info
                if si is not None and si.on_wait and len(si.on_wait) > 1 and all(
                    w.wait_mode == "sem-ge-imm" for w in si.on_wait
                ):
                    for w in si.on_wait[:-1]:
                        carrier = mybir.InstEventSemaphore(
                            name=f"I-{nc.next_id()}-waitsplit",
                            engine=inst.engine,
                            sync_info=mybir.SyncInfo(on_wait=[w], on_update=[]),
                        )
                        nc.inst_map[carrier.name] = carrier
                        out.append(carrier)
                    inst.sync_info = mybir.SyncInfo(
                        on_wait=[si.on_wait[-1]], on_update=si.on_update
                    )
                out.append(inst)
            blk.instructions[:] = out


def _get_nc():
    key = (N, D, C, NCORES)
    if key not in _NC_CACHE:
        _NC_CACHE[key] = _build_nc(*key)
    return _NC_CACHE[key]


def make_in_maps(embeddings, target, n=N, ncls=C, ncores=NCORES):
    b = n // ncores
    emb = np.ascontiguousarray(np.asarray(embeddings, dtype=np.float32))
    tgt = np.asarray(target).astype(np.int64) % ncls
    onehot = np.eye(ncls, dtype=np.float32)[tgt]  # [n, ncls]
    i128 = np.eye(128, dtype=np.float32)
    in_maps = []
    for c in range(ncores):
        xs = np.ascontiguousarray(np.roll(emb, -c * b, axis=0))
        ts = np.ascontiguousarray(np.roll(onehot, -c * b, axis=0))
        oha = np.ascontiguousarray(ts[:b].T)  # [ncls, b]
        in_maps.append({"x": xs, "tag": ts, "oha": oha, "i128": i128})
    return in_maps


def kernel(embeddings, target):
    from concourse.bass_utils import run_bass_kernel_spmd

    nc = _get_nc()
    in_maps = make_in_maps(embeddings, target)
    res = run_bass_kernel_spmd(nc, in_maps, list(range(NCORES))).results
    loss = np.concatenate([np.asarray(res[c]["loss"]).ravel() for c in range(NCORES)])
    return np.float32(loss.sum())



# revision 8
# speedup vs baseline: 35.4677x; 35.4677x over previous
"""Decoupled InfoNCE loss on 8 Trainium2 NeuronCores (Bass/Tile SPMD).

Math (reference):
    e = x / max(||x||, 1e-8);  sim = (e @ e.T) / 0.1
    pos = (t_i == t_j);  lse_neg = LSE_j(sim | not pos);  lse_pos = LSE_j(sim | pos & j != i)
    loss = sum_i (lse_neg_i - lse_pos_i)

Device strategy (per core c, anchors = rows [c*B, c*B+B)):
  * All logits sim/T lie in [-10, 10], so exp never overflows and the LSE
    max-subtraction can be dropped: lse = log(sum exp(sim/T)).
  * Inputs are row-rotated per core (np.roll) so each core's anchors are rows
    [0, B) of its own copy -> all 8 cores run one identical static program.
  * eT = transpose(sqrt(10) * e) is built on-chip in bf16 ([d, j] layout, d on
    partitions); sim-chunks [128 j, 512 i] come from two K=128 bf16 matmuls
    (1 PE cycle/row vs 4 for fp32).
  * Class masks are rank-64: with onehot tags M[cls, i] = sum_j 1[t_j==cls] *
    exp(sim_ji) (one extra bf16 matmul per chunk vs. the exp tile), then
      S_pos_incl[i] = M[t_i, i],  S_neg[i] = sum_cls M[cls, i] - M[t_i, i]
    computed exactly by elementwise one-hot select + ones-matmul column sums.
  * exp tiles are written in bf16; the diagonal exp(sim_ii) is extracted
    bit-exactly from the same bf16 tiles (first 8 j-blocks) so subtracting it
    from S_pos_incl cancels exactly despite bf16 rounding of the huge e^10.
  * Engine split: PE = transposes + sim + tag matmuls (all bf16); Act =
    square+row-sum (fused via accum_out), sqrt, exp, ln; DVE = row scaling
    (fp32 -> bf16), diag extract, phase-3 selects; Pool = PSUM->SBUF copies.
  * Output per core: per-anchor loss rows [B]; host concatenates and sums.
"""

import sys

if "/opt/trn_rl_repo" not in sys.path:
    sys.path.insert(0, "/opt/trn_rl_repo")

import numpy as np

N = 8192          # total rows
D = 256           # embedding dim
C = 64            # num classes
NCORES = 8
B = N // NCORES   # anchors per core
SQT = float(np.sqrt(10.0))  # sqrt(1/temperature); applied to both operands
EPS = 1e-8

_NC_CACHE = {}


def _build_nc(n=N, d=D, ncls=C, ncores=NCORES, reps=1):
    import concourse.bass as bass
    import concourse.mybir as mybir
    from concourse import tile

    f32 = mybir.dt.float32
    bf16 = mybir.dt.bfloat16
    Act = mybir.ActivationFunctionType
    AX = mybir.AxisListType

    b = n // ncores       # anchors per core
    nt = n // 128         # j tiles
    hb = d // 128         # 128-deep K steps per matmul
    nab = b // 512        # 512-wide anchor blocks
    ndj = b // 128        # j-blocks containing diagonal (first ndj blocks)

    nc = bass.Bass()
    x_d = nc.dram_tensor("x", [n, d], bf16, kind="ExternalInput")
    tag_d = nc.dram_tensor("tag", [n, ncls], bf16, kind="ExternalInput")
    oha_d = nc.dram_tensor("oha", [ncls, b], f32, kind="ExternalInput")
    i128_d = nc.dram_tensor("i128", [128, 128], f32, kind="ExternalInput")
    i128b_d = nc.dram_tensor("i128b", [128, 128], bf16, kind="ExternalInput")
    loss_d = nc.dram_tensor("loss", [1, b], f32, kind="ExternalOutput")

    with tile.TileContext(nc) as tc:
        with (
            tc.tile_pool(name="persist", bufs=1) as pp,
            tc.tile_pool(name="work", bufs=6) as wp,
            tc.tile_pool(name="expp", bufs=10) as ep,
            tc.tile_pool(name="c0p", bufs=6, space="PSUM") as c0p,
            tc.tile_pool(name="mp", bufs=1, space="PSUM") as mp,
        ):
            # ---- persistent SBUF ----
            eT = pp.tile([128, hb, n], bf16, tag="eT")
            tagS = pp.tile([128, nt, ncls], bf16, tag="tagS")
            ohaS = pp.tile([ncls, b], f32, tag="ohaS")
            i128 = pp.tile([128, 128], f32, tag="i128")
            i128b = pp.tile([128, 128], bf16, tag="i128b")
            ones = pp.tile([ncls, 1], f32, tag="ones")
            ddrow = pp.tile([1, b], f32, tag="ddrow")    # exp(sim_ii)
            sposr = pp.tile([1, b], f32, tag="sposr")    # S_pos incl diag
            snegr = pp.tile([1, b], f32, tag="snegr")    # S_neg
            lnegr = pp.tile([1, b], f32, tag="lnegr")
            lossr = pp.tile([1, b], f32, tag="lossr")

            nc.sync.dma_start(out=tagS[:], in_=tag_d.rearrange("(t p) c -> p t c", p=128))
            nc.sync.dma_start(out=ohaS[:], in_=oha_d[:])
            nc.sync.dma_start(out=i128[:], in_=i128_d[:])
            nc.sync.dma_start(out=i128b[:], in_=i128b_d[:])
            nc.vector.memset(ones[:], 1.0)

            # macc psum accumulators live across the whole j loop
            macc = [mp.tile([ncls, 512], f32, tag=f"m{ab}", name=f"macc{ab}")
                    for ab in range(nab)]

            # reps>1 repeats the whole computation in one NEFF; used only to
            # measure per-iteration HW time as a slope (dispatch overhead on
            # the axon path dwarfs a single run).
            for _rep in range(reps):
                _emit_body(nc, tile, mybir, locals())

    _split_multi_waits(nc)
    nc.finalize()
    return nc


def _emit_body(nc, tile, mybir, env):
    f32 = mybir.dt.float32
    bf16 = mybir.dt.bfloat16
    Act = mybir.ActivationFunctionType
    AX = mybir.AxisListType
    n, d, ncls, b = env["n"], env["d"], env["ncls"], env["b"]
    nt, hb, nab, ndj = env["nt"], env["hb"], env["nab"], env["ndj"]
    x_d, loss_d = env["x_d"], env["loss_d"]
    eT, tagS, ohaS, i128, i128b, ones = (env["eT"], env["tagS"], env["ohaS"],
                                         env["i128"], env["i128b"], env["ones"])
    ddrow, sposr, snegr, lnegr, lossr = (env["ddrow"], env["sposr"],
                                         env["snegr"], env["lnegr"],
                                         env["lossr"])
    wp, ep, c0p = env["wp"], env["ep"], env["c0p"]
    macc = env["macc"]

    if True:
        if True:
            # ---- phase 1 (per tile): normalize + scale + transpose into eT ----
            # Per 128-row tile: ss = sum(x^2) fused on Act (Square+accum_out);
            # inv = sqrt(10)/max(||row||, eps); xb = bf16(x * inv) on DVE; PE
            # transpose (1 cyc/row in bf16) into PSUM; Pool copies into eT.
            def phase1(t):
                xt = wp.tile([128, d], bf16, tag="xt", name="xt")
                nc.sync.dma_start(out=xt[:], in_=x_d[t * 128:(t + 1) * 128, :])
                sqs = wp.tile([128, d], f32, tag="sqs", name="sqs")
                ss = wp.tile([128, 1], f32, tag="ss", name="ss")
                nc.scalar.activation(sqs[:], xt[:], Act.Square, accum_out=ss[:])
                nrm = wp.tile([128, 1], f32, tag="nrm", name="nrm")
                nc.scalar.activation(nrm[:], ss[:], Act.Sqrt)
                nc.vector.tensor_scalar_max(nrm[:], nrm[:], EPS)
                inv = wp.tile([128, 1], f32, tag="inv", name="inv")
                nc.vector.reciprocal(inv[:], nrm[:])
                xb = wp.tile([128, d], bf16, tag="xb", name="xb")
                nc.vector.tensor_scalar(xb[:], xt[:], inv[:], SQT,
                                        op0=mybir.AluOpType.mult,
                                        op1=mybir.AluOpType.mult)
                for h in range(hb):
                    pt = c0p.tile([128, 128], bf16, tag="c0", name="pt")
                    nc.tensor.matmul(pt[:], xb[:, h * 128:(h + 1) * 128], i128b[:],
                                     is_transpose=True, start=True, stop=True)
                    nc.vector.tensor_copy(eT[:, h, t * 128:(t + 1) * 128], pt[:])

            # ---- phase 2: sim chunks -> exp -> class-sum matmuls ----
            live = {}  # (jb, ab) -> exp tile, consumed one jb later

            def consume(jb):
                exs = [live.pop((jb, ab)) for ab in range(nab)]
                for ab in range(nab):
                    nc.tensor.matmul(macc[ab][:], tagS[:, jb, :], exs[ab][:],
                                     start=(jb == 0), stop=(jb == nt - 1))
                if jb < ndj:
                    abd, off = (jb * 128) // 512, (jb * 128) % 512
                    dtmp = wp.tile([128, 128], f32, tag="dtmp")
                    nc.vector.tensor_mul(dtmp[:], exs[abd][:, off:off + 128], i128b[:])
                    dcol = wp.tile([128, 1], f32, tag="dcol")
                    nc.vector.reduce_sum(dcol[:], dtmp[:], axis=AX.X)
                    drow = c0p.tile([1, 512], f32, tag="c0", name="drow")
                    nc.tensor.matmul(drow[:, :128], dcol[:], i128[:],
                                     start=True, stop=True)
                    nc.vector.tensor_copy(ddrow[:, jb * 128:(jb + 1) * 128], drow[:, :128])

            LAG = 2  # jb's of slack between C0 production and M' consumption

            def sim_block(jb):
                # h-outer order: consecutive matmuls share lhsT (one weight
                # load per (jb, h) instead of per (jb, h, ab)); accumulation
                # groups of the nab psum banks interleave, which the PE
                # handles per-bank (skip the group contiguity check).
                c0s = []
                for ab in range(nab):
                    c0 = c0p.tile([128, 512], f32, tag="c0", name=f"c0_{jb}_{ab}")
                    c0s.append(c0)
                for h in range(hb):
                    for ab in range(nab):
                        nc.tensor.matmul(c0s[ab][:],
                                         eT[:, h, jb * 128:(jb + 1) * 128],
                                         eT[:, h, ab * 512:(ab + 1) * 512],
                                         start=(h == 0), stop=(h == hb - 1),
                                         skip_group_check=True)
                for ab in range(nab):
                    ex = ep.tile([128, 512], bf16, tag="exp", name=f"ex_{jb}_{ab}")
                    nc.scalar.activation(ex[:], c0s[ab][:], Act.Exp)
                    live[(jb, ab)] = ex
                if jb >= LAG:
                    consume(jb - LAG)

            # Interleave phase-1 tiles with sim blocks so the PE never idles
            # waiting for the full input DMA: sim(jb) needs eT tiles 0..7
            # (anchor columns) plus tile jb, so it is emitted right after
            # phase1(jb + 8).
            PRE = ndj  # anchor tiles that must exist before sim(0)
            for t in range(nt):
                phase1(t)
                if t >= PRE:
                    sim_block(t - PRE)
            for jb in range(nt - PRE, nt):
                sim_block(jb)
            for jb in range(nt - LAG, nt):
                consume(jb)

            # ---- phase 3: select own-class / other-class sums ----
            for ab in range(nab):
                msb = wp.tile([ncls, 512], f32, tag="msb")
                nc.vector.tensor_copy(msb[:], macc[ab][:])
                x1 = wp.tile([ncls, 512], f32, tag="x1")
                nc.vector.tensor_mul(x1[:], msb[:], ohaS[:, ab * 512:(ab + 1) * 512])
                x2 = wp.tile([ncls, 512], f32, tag="x2")
                nc.vector.tensor_sub(x2[:], msb[:], x1[:])
                s1 = c0p.tile([1, 512], f32, tag="c0", name="s1")
                nc.tensor.matmul(s1[:], ones[:], x1[:], start=True, stop=True)
                nc.vector.tensor_copy(sposr[:, ab * 512:(ab + 1) * 512], s1[:])
                s2 = c0p.tile([1, 512], f32, tag="c0", name="s2")
                nc.tensor.matmul(s2[:], ones[:], x2[:], start=True, stop=True)
                nc.vector.tensor_copy(snegr[:, ab * 512:(ab + 1) * 512], s2[:])

            # ---- phase 4: loss rows ----
            nc.vector.tensor_sub(sposr[:], sposr[:], ddrow[:])
            nc.scalar.activation(lnegr[:], snegr[:], Act.Ln)
            nc.scalar.activation(lossr[:], sposr[:], Act.Ln)
            nc.vector.tensor_sub(lossr[:], lnegr[:], lossr[:])
            nc.sync.dma_start(out=loss_d[:], in_=lossr[:])


def _split_multi_waits(nc):
    """Move extra semaphore waits onto standalone EventSemaphore carriers.

    The pinned walrus build only has one sync-wait slot per engine
    instruction ("Too many sync wait commands"), while the Tile scheduler
    happily attaches several. All waits here are monotonic sem-ge-imm, so
    waiting sequentially on the same engine is equivalent to waiting on the
    conjunction.
    """
    import concourse.mybir as mybir

    for fn in nc.m.functions:
        for blk in fn.blocks:
            out = []
            for inst in blk.instructions:
                si = inst.sync_info
                if si is not None and si.on_wait and len(si.on_wait) > 1 and all(
                    w.wait_mode == "sem-ge-imm" for w in si.on_wait
                ):
                    for w in si.on_wait[:-1]:
                        carrier = mybir.InstEventSemaphore(
                            name=f"I-{nc.next_id()}-waitsplit",
                            engine=inst.engine,
                            sync_info=mybir.SyncInfo(on_wait=[w], on_update=[]),
                        )
                        nc.inst_map[carrier.name] = carrier
                        out.append(carrier)
                    inst.sync_info = mybir.SyncInfo(
                        on_wait=[si.on_wait[-1]], on_update=si.on_update
                    )
                out.append(inst)
            blk.instructions[:] = out


def _get_nc():
    key = (N, D, C, NCORES)
    if key not in _NC_CACHE:
        _NC_CACHE[key] = _build_nc(*key)
    return _NC_CACHE[key]


def make_in_maps(embeddings, target, n=N, ncls=C, ncores=NCORES):
    import ml_dtypes

    b = n // ncores
    emb = np.asarray(embeddings, dtype=np.float32).astype(ml_dtypes.bfloat16)
    tgt = np.asarray(target).astype(np.int64) % ncls
    onehot = np.eye(ncls, dtype=np.float32)[tgt]  # [n, ncls]
    onehot_b = onehot.astype(ml_dtypes.bfloat16)
    i128 = np.eye(128, dtype=np.float32)
    i128b = i128.astype(ml_dtypes.bfloat16)
    in_maps = []
    for c in range(ncores):
        xs = np.ascontiguousarray(np.roll(emb, -c * b, axis=0))
        ts = np.ascontiguousarray(np.roll(onehot_b, -c * b, axis=0))
        oha = np.ascontiguousarray(np.roll(onehot, -c * b, axis=0)[:b].T)  # [ncls, b]
        in_maps.append({"x": xs, "tag": ts, "oha": oha, "i128": i128, "i128b": i128b})
    return in_maps


def kernel(embeddings, target):
    from concourse.bass_utils import run_bass_kernel_spmd

    nc = _get_nc()
    in_maps = make_in_maps(embeddings, target)
    res = run_bass_kernel_spmd(nc, in_maps, list(range(NCORES))).results
    loss = np.concatenate([np.asarray(res[c]["loss"]).ravel() for c in range(NCORES)])
    return np.float32(loss.sum())
